# revision 1
# baseline (speedup 1.0000x reference)
"""Self-contained Trainium2 Bass kernel for the 2-layer GCN encoder.

kernel(**inputs) takes FULL inputs (features [100000,128] f32,
edge_index [2,1600000] int, edge_weight [1600000] f32, W1,b1,a1,W2,b2,a2)
and returns the FULL [100000,128] f32 output, running on 8 NeuronCores.
"""
import os
os.environ.setdefault("NEURON_RT_RESET_CORES", "1")

_DOC = """GCN 2-layer encoder on 8 Trainium2 NeuronCores.

Strategy (dst-sharded message passing, replicated weights):
  - nodes sharded across 8 cores (SH each, padded to NT*128)
  - per layer: GEMM on own shard -> AllGather h-table -> dst-sharded
    aggregation via dma_gather (h rows) + one-hot*norm S matrices built with
    a fused is_equal/mult tensor_scalar + PE matmul segment-sum in PSUM
  - norm (sym-normalized edge weights incl self loops) computed on host;
    edges grouped per (dst-super-tile, src-pass) with per-(tile,pass) block
    counts baked into the program (max over cores -> SPMD-uniform schedule)
  - layer 1 uses matmul form out[f,d] (lhsT=msgs, rhs=S) so its output feeds
    GEMM2 as lhsT directly; layer 2 uses form out[d,f] (lhsT=S, rhs=msgs) so
    output rows DMA straight to HBM.
"""

import sys
sys.path.insert(0, "/opt/trn_rl_repo")
import numpy as np
from concourse import bacc, mybir, library_config
from concourse.bass_utils import run_bass_kernel_spmd

F32 = mybir.dt.float32
I16 = mybir.dt.int16

C = 8            # cores
D = 128          # feature dim
NPASS = 4        # src-range passes (int16 gather indices)
SUPER = 4        # dst tiles per super-tile (one PSUM bank per tile)
MAXIDX = 1024    # max indices per dma_gather call (HW ring limit)
RING = 32        # S-tile ring slots
GRP = 8          # S-ring sync granularity (blocks)


def _schedule(N, src, dst, norm):
    """Group edges into an SPMD-uniform static schedule."""
    SH = N // C
    NT = (SH + 127) // 128          # dst tiles per core
    SHP = NT * 128
    NS = (NT + SUPER - 1) // SUPER  # super tiles
    TBL = SHP * C
    PR = (TBL + NPASS - 1) // NPASS
    PR = ((PR + 127) // 128) * 128  # pass rows (aligned)
    assert PR <= 32767

    core = dst // SH
    dloc = dst - core * SH
    tile = dloc // 128
    srel = (dloc % 128).astype(np.float32)
    sup = tile // SUPER
    tin = tile % SUPER              # tile index within super
    tbl = (src // SH) * SHP + (src % SH)
    pas = tbl // PR
    prel = (tbl % PR).astype(np.int16)

    ntin = np.minimum(SUPER, NT - np.arange(NS) * SUPER)  # tiles in super s

    key = ((core * NS + sup) * NPASS + pas) * SUPER + tin
    cnt = np.bincount(key, minlength=C * NS * NPASS * SUPER).reshape(
        C, NS, NPASS, SUPER
    )
    B = np.maximum(1, -(-cnt.max(axis=0) // 128))         # [NS, NPASS, SUPER]
    for s in range(NS):
        B[s, :, ntin[s]:] = 0
    Bf = B.reshape(-1)
    bbase = np.concatenate([[0], np.cumsum(Bf)]).astype(np.int64)
    TB = int(bbase[-1])                                   # blocks per layer

    # per-edge slot: blockbase(s,p,t)*128 + rank within (core,s,p,t) group
    ekey = (sup * NPASS + pas) * SUPER + tin
    order = np.lexsort((ekey, core))
    sc, se = core[order], ekey[order]
    gkey = sc * (NS * NPASS * SUPER) + se
    _, first = np.unique(gkey, return_index=True)
    starts = np.zeros(len(gkey), dtype=np.int64)
    starts[first] = first
    starts = np.maximum.accumulate(starts)
    rank = np.arange(len(gkey)) - starts
    slot = bbase[se] * 128 + rank

    idx_f = np.zeros((C, TB * 128), dtype=np.int16)
    dr_f = np.zeros((C, TB * 128), dtype=np.float32)
    nm_f = np.zeros((C, TB * 128), dtype=np.float32)
    idx_f[sc, slot] = prel[order]
    dr_f[sc, slot] = srel[order]
    nm_f[sc, slot] = norm[order]

    # gather sub-calls: per (s,p), chunks of <= MAXIDX/128 blocks
    calls = []   # (s, p, fb(layer-block), nbl, queue, qseq)
    qcnt = [0, 0, 0, 0]
    for s in range(NS):
        for p in range(NPASS):
            nb = int(B[s, p].sum())
            fb0 = int(bbase[(s * NPASS + p) * SUPER])
            off = 0
            while off < nb:
                n = min(nb - off, MAXIDX // 128)
                q = len(calls) % 4
                qcnt[q] += 1
                calls.append((s, p, fb0 + off, n, q, qcnt[q]))
                off += n
    maxblk = max(int(B[s, p].sum()) for s in range(NS) for p in range(NPASS))

    # block -> sp map and cumulative blocks through sp
    blk_sp = np.zeros(TB, dtype=np.int64)
    cum_sp = {}
    acc = 0
    for s in range(NS):
        for p in range(NPASS):
            sp = s * NPASS + p
            fb0 = int(bbase[sp * SUPER])
            nb = int(B[s, p].sum())
            blk_sp[fb0:fb0 + nb] = sp
            acc += nb
            cum_sp[sp] = acc

    # block -> (tile-in-super, block-ofs, is_first, is_last)
    blk_tile = np.zeros(TB, dtype=np.int64)
    blk_first = np.zeros(TB, dtype=bool)
    blk_last = np.zeros(TB, dtype=bool)
    for s in range(NS):
        for p in range(NPASS):
            for t in range(int(ntin[s])):
                b0 = int(bbase[(s * NPASS + p) * SUPER + t])
                nb = int(B[s, p, t])
                blk_tile[b0:b0 + nb] = t
                if p == 0:
                    blk_first[b0] = True
                if p == NPASS - 1:
                    blk_last[b0 + nb - 1] = True

    return dict(
        N=N, SH=SH, NT=NT, SHP=SHP, NS=NS, TBL=TBL, PR=PR, TB=TB,
        B=B, bbase=bbase, ntin=ntin, calls=calls, maxblk=maxblk,
        blk_sp=blk_sp, cum_sp=cum_sp, blk_tile=blk_tile,
        blk_first=blk_first, blk_last=blk_last,
        qtot=[qcnt[0], qcnt[1], qcnt[2], qcnt[3]],
    ), idx_f, dr_f, nm_f


def _wrap_idx(idx_f):
    """[C, TB*128] int16 -> [C, 128, TB*8] wrapped-16 layout, replicated x8."""
    Cn, L = idx_f.shape
    t = idx_f.reshape(Cn, L // 16, 16).transpose(0, 2, 1)
    return np.tile(t, (1, 8, 1)).copy()


def build_program(meta):
    NT, SHP, NS, TBL, PR, TB = (
        meta["NT"], meta["SHP"], meta["NS"], meta["TBL"], meta["PR"], meta["TB"]
    )
    B, bbase, ntin, calls, maxblk = (
        meta["B"], meta["bbase"], meta["ntin"], meta["calls"], meta["maxblk"]
    )
    blk_sp, cum_sp = meta["blk_sp"], meta["cum_sp"]
    blk_tile, blk_first, blk_last = (
        meta["blk_tile"], meta["blk_first"], meta["blk_last"]
    )
    qtot = meta["qtot"]
    NSP = NS * NPASS
    NI16 = TB * 8
    STG = max(SHP, 2 * maxblk * 128)

    def gcum(spk):  # cumulative blocks through global sp-call spk
        return (spk // NSP) * TB + cum_sp[spk % NSP]

    nc = bacc.Bacc("TRN2", debug=False, num_swdge_queues=4)
    featT = nc.declare_dram_parameter("featT", [128, SHP], F32, isOutput=False)
    idxs = nc.declare_dram_parameter("idxs", [128, NI16], I16, isOutput=False)
    drel = nc.declare_dram_parameter("drel", [128, TB], F32, isOutput=False)
    nrm = nc.declare_dram_parameter("nrm", [128, TB], F32, isOutput=False)
    iota = nc.declare_dram_parameter("iota", [128, 128], F32, isOutput=False)
    w1 = nc.declare_dram_parameter("w1", [128, 128], F32, isOutput=False)
    w2 = nc.declare_dram_parameter("w2", [128, 128], F32, isOutput=False)
    b1 = nc.declare_dram_parameter("b1", [128, 1], F32, isOutput=False)
    a1 = nc.declare_dram_parameter("a1", [128, 1], F32, isOutput=False)
    b2bc = nc.declare_dram_parameter("b2bc", [128, 128], F32, isOutput=False)
    a2bc = nc.declare_dram_parameter("a2bc", [128, 128], F32, isOutput=False)
    out = nc.declare_dram_parameter("out", [SHP, D], F32, isOutput=True)

    h_bounce = [nc.dram_tensor(f"h{l}_bounce", [SHP, D], F32) for l in (1, 2)]
    h_table = [
        nc.dram_tensor(f"h{l}_table", [TBL, D], F32, addr_space="Shared")
        for l in (1, 2)
    ]

    from contextlib import ExitStack
    with ExitStack() as ctx:
        ent = ctx.enter_context
        xbuf = ent(nc.sbuf_tensor("xbuf", [128, SHP], F32))
        stage = ent(nc.sbuf_tensor("stage", [128, STG], F32))
        idx_sb = ent(nc.sbuf_tensor("idx_sb", [128, NI16], I16))
        drel_sb = ent(nc.sbuf_tensor("drel_sb", [128, TB], F32))
        nrm_sb = ent(nc.sbuf_tensor("nrm_sb", [128, TB], F32))
        sring = ent(nc.sbuf_tensor("sring", [128, RING, 128], F32))
        iota_sb = ent(nc.sbuf_tensor("iota_sb", [128, 128], F32))
        w1_sb = ent(nc.sbuf_tensor("w1_sb", [128, 128], F32))
        w2_sb = ent(nc.sbuf_tensor("w2_sb", [128, 128], F32))
        b1_sb = ent(nc.sbuf_tensor("b1_sb", [128, 1], F32))
        a1_sb = ent(nc.sbuf_tensor("a1_sb", [128, 1], F32))
        b2bc_sb = ent(nc.sbuf_tensor("b2bc_sb", [128, 128], F32))
        a2bc_sb = ent(nc.sbuf_tensor("a2bc_sb", [128, 128], F32))
        tpos = ent(nc.sbuf_tensor("tpos", [128, 128], F32))
        tneg = ent(nc.sbuf_tensor("tneg", [128, 128], F32))
        ps_all = ent(nc.psum_tensor("ps_all", [128, 8, 512], F32))
        s_load = ent(nc.semaphore("s_load"))
        s_gat = [ent(nc.semaphore(f"s_ga{i}")) for i in range(8)]
        s_dve = ent(nc.semaphore("s_dve"))
        s_pe = ent(nc.semaphore("s_pe"))
        s_peg = ent(nc.semaphore("s_peg"))
        s_dveg = ent(nc.semaphore("s_dveg"))
        s_post = ent(nc.semaphore("s_post"))
        s_store = ent(nc.semaphore("s_store"))
        s_cc = ent(nc.semaphore("s_cc"))
        s_ch = ent(nc.semaphore("s_ch"))
        block = ent(nc.Block())

        def agg_ps(s, t):
            return ps_all[:, (s % 2) * 4 + t, 0:128]

        def gemm_ps(t):
            return ps_all[:, 4 + (t % 2), 0:128]
        stage3 = stage[:, : 2 * maxblk * 128].rearrange(
            "p (b f) -> p b f", f=128
        )

        def msg_ap(layer, gl):
            sp = int(blk_sp[gl])
            buf = (layer * NSP + sp) % 2
            loc = gl - int(bbase[sp * SUPER])
            return stage3[:, buf * maxblk + loc, :]

        chain = [0]  # DVE same-engine hazard chain counter
        pc = [0]     # completed posts (guards tpos/tneg WAR across tiles)

        def post(vector, layer, s):
            vector.wait_ge(s_pe, TB * layer + cum_sp[s * NPASS + NPASS - 1])
            for t in range(int(ntin[s])):
                gt = s * SUPER + t
                ps = agg_ps(s, t)
                xo = xbuf[:, gt * 128: (gt + 1) * 128]
                if pc[0] > 0:
                    vector.wait_ge(s_post, pc[0])
                if layer == 0:
                    vector.tensor_scalar(
                        tpos[:], ps, b1_sb[:, 0:1], 0.0,
                        op0=mybir.AluOpType.add, op1=mybir.AluOpType.max,
                    )
                    vector.tensor_scalar(
                        tneg[:], ps, b1_sb[:, 0:1], 0.0,
                        op0=mybir.AluOpType.add, op1=mybir.AluOpType.min,
                    ).then_inc(s_ch, 1)
                    chain[0] += 1
                    vector.wait_ge(s_ch, chain[0])
                    vector.tensor_scalar(
                        tneg[:], tneg[:], a1_sb[:, 0:1], None,
                        op0=mybir.AluOpType.mult,
                    ).then_inc(s_ch, 1)
                    chain[0] += 1
                    vector.wait_ge(s_ch, chain[0])
                    vector.tensor_tensor(
                        xo, tpos[:], tneg[:], op=mybir.AluOpType.add
                    ).then_inc(s_post, 1)
                    pc[0] += 1
                else:
                    vector.tensor_tensor(
                        tpos[:], ps, b2bc_sb[:], op=mybir.AluOpType.add
                    ).then_inc(s_ch, 1)
                    chain[0] += 1
                    vector.wait_ge(s_ch, chain[0])
                    vector.tensor_scalar(
                        tneg[:], tpos[:], 0.0, None, op0=mybir.AluOpType.min
                    ).then_inc(s_ch, 1)
                    chain[0] += 1
                    vector.wait_ge(s_ch, chain[0])
                    vector.tensor_scalar(
                        tpos[:], tpos[:], 0.0, None, op0=mybir.AluOpType.max
                    )
                    vector.tensor_tensor(
                        tneg[:], tneg[:], a2bc_sb[:], op=mybir.AluOpType.mult
                    ).then_inc(s_ch, 1)
                    chain[0] += 1
                    vector.wait_ge(s_ch, chain[0])
                    vector.tensor_tensor(
                        xo, tpos[:], tneg[:], op=mybir.AluOpType.add
                    ).then_inc(s_post, 1)
                    pc[0] += 1

        @block.sync
        def _(sync):
            for ap_d, ap_s in (
                (iota_sb[:], iota[:]), (w1_sb[:], w1[:]), (w2_sb[:], w2[:]),
                (b1_sb[:], b1[:]), (a1_sb[:], a1[:]),
                (b2bc_sb[:], b2bc[:]), (a2bc_sb[:], a2bc[:]),
                (idx_sb[:], idxs[:]), (drel_sb[:], drel[:]),
                (nrm_sb[:], nrm[:]), (xbuf[:, :SHP], featT[:]),
            ):
                sync.dma_start(out=ap_d, in_=ap_s).then_inc(s_load, 16)
            sync.wait_ge(s_dveg, NT)
            sync.dma_start(
                out=h_bounce[0].ap().rearrange("(t p) f -> p t f", p=128),
                in_=stage[:, : NT * 128].rearrange("p (t f) -> p t f", f=128),
            ).then_inc(s_store, 16)
            sync.wait_ge(s_dveg, 2 * NT)
            sync.dma_start(
                out=h_bounce[1].ap().rearrange("(t p) f -> p t f", p=128),
                in_=stage[:, : NT * 128].rearrange("p (t f) -> p t f", f=128),
            ).then_inc(s_store, 16)
            sync.wait_ge(s_post, 2 * NT)
            sync.dma_start(
                out=out.ap().rearrange("(t p) f -> p t f", p=128),
                in_=xbuf[:, : NT * 128].rearrange("p (t f) -> p t f", f=128),
            ).then_inc(s_store, 16)
            sync.wait_ge(s_store, 48)

        @block.gpsimd
        def _(gpsimd):
            gpsimd.load_library(library_config.mlp)
            for layer in range(2):
                gpsimd.wait_ge(s_store, 16 * (layer + 1))
                gpsimd.collective_compute(
                    "AllGather",
                    mybir.AluOpType.bypass,
                    replica_groups=[list(range(C))],
                    ins=[h_bounce[layer][:]],
                    outs=[h_table[layer][:]],
                ).then_inc(s_cc)
                gpsimd.wait_ge(s_cc, layer + 1)
                for k, (s, p, fb, nbl, q, qs) in enumerate(calls):
                    sp = s * NPASS + p
                    spk = layer * NSP + sp
                    kk = layer * len(calls) + k
                    if spk >= 2 and fb == int(bbase[sp * SUPER]):
                        gpsimd.wait_ge(s_pe, gcum(spk - 2))
                    buf = spk % 2
                    loc = fb - int(bbase[sp * SUPER])
                    gpsimd.dma_gather(
                        stage3[:, buf * maxblk + loc: buf * maxblk + loc + nbl, :],
                        h_table[layer][p * PR: min((p + 1) * PR, TBL), :],
                        idx_sb[:, fb * 8: (fb + nbl) * 8],
                        nbl * 128,
                        nbl * 128,
                        D,
                        queue_num=q,
                    ).then_inc(s_gat[kk % 8], 16)

        @block.vector
        def _(vector):
            vector.wait_ge(s_load, 176)
            for t in range(NT):
                vector.wait_ge(s_peg, t + 1)
                vector.tensor_copy(
                    stage[:, t * 128: (t + 1) * 128], gemm_ps(t)
                ).then_inc(s_dveg, 1)
            for layer in range(2):
                g0 = TB * layer
                for s in range(NS):
                    for p in range(NPASS):
                        sp = s * NPASS + p
                        fb0 = int(bbase[sp * SUPER])
                        nb = int(B[s, p].sum())
                        for gl in range(fb0, fb0 + nb):
                            g = g0 + gl
                            if gl % GRP == 0 and g >= RING:
                                vector.wait_ge(s_pe, g - RING + GRP)
                            vector.tensor_scalar(
                                sring[:, g % RING, :],
                                iota_sb[:],
                                drel_sb[:, gl: gl + 1],
                                nrm_sb[:, gl: gl + 1],
                                op0=mybir.AluOpType.is_equal,
                                op1=mybir.AluOpType.mult,
                            ).then_inc(s_dve, 1)
                    if s >= 1:
                        post(vector, layer, s - 1)
                post(vector, layer, NS - 1)
                if layer == 0:
                    for t in range(NT):
                        vector.wait_ge(s_peg, NT + t + 1)
                        vector.tensor_copy(
                            stage[:, t * 128: (t + 1) * 128], gemm_ps(t)
                        ).then_inc(s_dveg, 1)

        @block.tensor
        def _(tensor):
            tensor.wait_ge(s_load, 176)
            for t in range(NT):
                if t >= 2:
                    tensor.wait_ge(s_dveg, t - 1)
                tensor.matmul(
                    gemm_ps(t),
                    xbuf[:, t * 128: (t + 1) * 128],
                    w1_sb[:],
                    start=True, stop=True,
                ).then_inc(s_peg, 1)
            for layer in range(2):
                g0 = TB * layer
                for k, (s, p, fb, nbl, q, qs) in enumerate(calls):
                    kk = layer * len(calls) + k
                    tensor.wait_ge(s_gat[kk % 8], 16 * (kk // 8 + 1))
                    for gl in range(fb, fb + nbl):
                        g = g0 + gl
                        if gl % GRP == 0:
                            tensor.wait_ge(s_dve, min(g + GRP, g0 + TB))
                        t = int(blk_tile[gl])
                        ps = agg_ps(s, t)
                        m = msg_ap(layer, gl)
                        sr = sring[:, g % RING, :]
                        if layer == 0:
                            mm = tensor.matmul(
                                ps, m, sr,
                                start=bool(blk_first[gl]),
                                stop=bool(blk_last[gl]),
                                skip_group_check=True,
                            )
                        else:
                            mm = tensor.matmul(
                                ps, sr, m,
                                start=bool(blk_first[gl]),
                                stop=bool(blk_last[gl]),
                                skip_group_check=True,
                            )
                        mm.then_inc(s_pe, 1)
                if layer == 0:
                    for t in range(NT):
                        if t == 0:
                            tensor.wait_ge(s_post, NT)
                        if t >= 2:
                            tensor.wait_ge(s_dveg, NT + t - 1)
                        tensor.matmul(
                            gemm_ps(t),
                            xbuf[:, t * 128: (t + 1) * 128],
                            w2_sb[:],
                            start=True, stop=True,
                        ).then_inc(s_peg, 1)

    nc.compile()
    return nc


def prepare(features, edge_index, edge_weight, W1, b1, a1, W2, b2, a2):
    N, Dd = features.shape
    assert Dd == D
    src = np.asarray(edge_index[0], dtype=np.int64)
    dst = np.asarray(edge_index[1], dtype=np.int64)
    w = np.asarray(edge_weight, dtype=np.float32)

    deg = (np.bincount(dst, weights=w.astype(np.float64), minlength=N) + 1.0)
    dis = (1.0 / np.sqrt(deg)).astype(np.float32)
    norm = dis[src] * w * dis[dst]
    allsrc = np.concatenate([src, np.arange(N, dtype=np.int64)])
    alldst = np.concatenate([dst, np.arange(N, dtype=np.int64)])
    allnorm = np.concatenate([norm, (dis * dis).astype(np.float32)])

    meta, idx_f, dr_f, nm_f = _schedule(N, allsrc, alldst, allnorm)
    SH, SHP, TB = meta["SH"], meta["SHP"], meta["TB"]

    idx_w = _wrap_idx(idx_f)
    dr_w = dr_f.reshape(C, TB, 128).transpose(0, 2, 1).copy()
    nm_w = nm_f.reshape(C, TB, 128).transpose(0, 2, 1).copy()

    featT = np.zeros((C, 128, SHP), dtype=np.float32)
    fpad = np.asarray(features, dtype=np.float32)
    for c in range(C):
        featT[c, :, :SH] = fpad[c * SH:(c + 1) * SH].T

    iota = np.tile(np.arange(128, dtype=np.float32)[None, :], (128, 1))
    in_maps = []
    for c in range(C):
        in_maps.append(dict(
            featT=featT[c], idxs=idx_w[c], drel=dr_w[c], nrm=nm_w[c],
            iota=iota,
            w1=np.asarray(W1, np.float32), w2=np.asarray(W2, np.float32),
            b1=np.asarray(b1, np.float32).reshape(128, 1),
            a1=np.asarray(a1, np.float32).reshape(128, 1),
            b2bc=np.tile(np.asarray(b2, np.float32)[None, :], (128, 1)),
            a2bc=np.tile(np.asarray(a2, np.float32)[None, :], (128, 1)),
        ))
    return meta, in_maps


def kernel(features, edge_index, edge_weight, W1, b1, a1, W2, b2, a2):
    meta, in_maps = prepare(
        features, edge_index, edge_weight, W1, b1, a1, W2, b2, a2
    )
    nc = build_program(meta)
    res = run_bass_kernel_spmd(nc, in_maps, core_ids=list(range(C))).results
    SH = meta["SH"]
    return np.concatenate([r["out"][:SH] for r in res], axis=0)



# revision 4
# speedup vs baseline: 1.0116x; 1.0116x over previous
"""Self-contained Trainium2 Bass kernel for the 2-layer GCN encoder.

kernel(**inputs) takes FULL inputs (features [100000,128] f32,
edge_index [2,1600000] int, edge_weight [1600000] f32, W1,b1,a1,W2,b2,a2)
and returns the FULL [100000,128] f32 output, running on 8 NeuronCores.
"""
import os
os.environ.setdefault("NEURON_RT_RESET_CORES", "1")

_DOC = """GCN 2-layer encoder on 8 Trainium2 NeuronCores.

Strategy (dst-sharded message passing, replicated weights):
  - nodes sharded across 8 cores (SH each, padded to NT*128)
  - per layer: GEMM on own shard -> AllGather h-table -> dst-sharded
    aggregation via dma_gather (h rows) + one-hot*norm S matrices built with
    a fused is_equal/mult tensor_scalar + PE matmul segment-sum in PSUM
  - norm (sym-normalized edge weights incl self loops) computed on host;
    edges grouped per (dst-super-tile, src-pass) with per-(tile,pass) block
    counts baked into the program (max over cores -> SPMD-uniform schedule)
  - layer 1 uses matmul form out[f,d] (lhsT=msgs, rhs=S) so its output feeds
    GEMM2 as lhsT directly; layer 2 uses form out[d,f] (lhsT=S, rhs=msgs) so
    output rows DMA straight to HBM.
"""

import sys
sys.path.insert(0, "/opt/trn_rl_repo")
import numpy as np
from ml_dtypes import bfloat16
from concourse import bacc, mybir, library_config
from concourse.bass_utils import run_bass_kernel_spmd

F32 = mybir.dt.float32
BF16 = mybir.dt.bfloat16
I16 = mybir.dt.int16

C = 8            # cores
D = 128          # feature dim
NPASS = 4        # src-range passes (int16 gather indices)
SUPER = 4        # dst tiles per super-tile (one PSUM bank per tile)
MAXIDX = 1024    # max indices per dma_gather call (HW ring limit)
RING = 32        # S-tile ring slots
GRP = 8          # S-ring sync granularity (blocks)


def _schedule(N, src, dst, norm):
    """Group edges into an SPMD-uniform static schedule."""
    SH = N // C
    NT = (SH + 127) // 128          # dst tiles per core
    SHP = NT * 128
    NS = (NT + SUPER - 1) // SUPER  # super tiles
    TBL = SHP * C
    PR = (TBL + NPASS - 1) // NPASS
    PR = ((PR + 127) // 128) * 128  # pass rows (aligned)
    assert PR <= 32767

    core = dst // SH
    dloc = dst - core * SH
    tile = dloc // 128
    srel = (dloc % 128).astype(np.float32)
    sup = tile // SUPER
    tin = tile % SUPER              # tile index within super
    tbl = (src // SH) * SHP + (src % SH)
    pas = tbl // PR
    prel = (tbl % PR).astype(np.int16)

    ntin = np.minimum(SUPER, NT - np.arange(NS) * SUPER)  # tiles in super s

    key = ((core * NS + sup) * NPASS + pas) * SUPER + tin
    cnt = np.bincount(key, minlength=C * NS * NPASS * SUPER).reshape(
        C, NS, NPASS, SUPER
    )
    B = np.maximum(1, -(-cnt.max(axis=0) // 128))         # [NS, NPASS, SUPER]
    for s in range(NS):
        B[s, :, ntin[s]:] = 0
    Bf = B.reshape(-1)
    bbase = np.concatenate([[0], np.cumsum(Bf)]).astype(np.int64)
    TB = int(bbase[-1])                                   # blocks per layer

    # per-edge slot: blockbase(s,p,t)*128 + rank within (core,s,p,t) group
    ekey = (sup * NPASS + pas) * SUPER + tin
    order = np.lexsort((ekey, core))
    sc, se = core[order], ekey[order]
    gkey = sc * (NS * NPASS * SUPER) + se
    _, first = np.unique(gkey, return_index=True)
    starts = np.zeros(len(gkey), dtype=np.int64)
    starts[first] = first
    starts = np.maximum.accumulate(starts)
    rank = np.arange(len(gkey)) - starts
    slot = bbase[se] * 128 + rank

    idx_f = np.zeros((C, TB * 128), dtype=np.int16)
    dr_f = np.zeros((C, TB * 128), dtype=np.float32)
    nm_f = np.zeros((C, TB * 128), dtype=np.float32)
    idx_f[sc, slot] = prel[order]
    dr_f[sc, slot] = srel[order]
    nm_f[sc, slot] = norm[order]

    # gather sub-calls: per (s,p), chunks of <= MAXIDX/128 blocks
    calls = []   # (s, p, fb(layer-block), nbl, queue, qseq)
    qcnt = [0, 0, 0, 0]
    for s in range(NS):
        for p in range(NPASS):
            nb = int(B[s, p].sum())
            fb0 = int(bbase[(s * NPASS + p) * SUPER])
            off = 0
            while off < nb:
                n = min(nb - off, MAXIDX // 128)
                q = len(calls) % 4
                qcnt[q] += 1
                calls.append((s, p, fb0 + off, n, q, qcnt[q]))
                off += n
    maxblk = max(int(B[s, p].sum()) for s in range(NS) for p in range(NPASS))

    # block -> sp map and cumulative blocks through sp
    blk_sp = np.zeros(TB, dtype=np.int64)
    cum_sp = {}
    acc = 0
    for s in range(NS):
        for p in range(NPASS):
            sp = s * NPASS + p
            fb0 = int(bbase[sp * SUPER])
            nb = int(B[s, p].sum())
            blk_sp[fb0:fb0 + nb] = sp
            acc += nb
            cum_sp[sp] = acc

    # block -> (tile-in-super, block-ofs, is_first, is_last)
    blk_tile = np.zeros(TB, dtype=np.int64)
    blk_first = np.zeros(TB, dtype=bool)
    blk_last = np.zeros(TB, dtype=bool)
    for s in range(NS):
        for p in range(NPASS):
            for t in range(int(ntin[s])):
                b0 = int(bbase[(s * NPASS + p) * SUPER + t])
                nb = int(B[s, p, t])
                blk_tile[b0:b0 + nb] = t
                if p == 0:
                    blk_first[b0] = True
                if p == NPASS - 1:
                    blk_last[b0 + nb - 1] = True

    return dict(
        N=N, SH=SH, NT=NT, SHP=SHP, NS=NS, TBL=TBL, PR=PR, TB=TB,
        B=B, bbase=bbase, ntin=ntin, calls=calls, maxblk=maxblk,
        blk_sp=blk_sp, cum_sp=cum_sp, blk_tile=blk_tile,
        blk_first=blk_first, blk_last=blk_last,
        qtot=[qcnt[0], qcnt[1], qcnt[2], qcnt[3]],
    ), idx_f, dr_f, nm_f


def build_program(meta):
    NT, SHP, NS, TBL, PR, TB = (
        meta["NT"], meta["SHP"], meta["NS"], meta["TBL"], meta["PR"], meta["TB"]
    )
    B, bbase, ntin, calls, maxblk = (
        meta["B"], meta["bbase"], meta["ntin"], meta["calls"], meta["maxblk"]
    )
    blk_sp, cum_sp = meta["blk_sp"], meta["cum_sp"]
    blk_tile, blk_first, blk_last = (
        meta["blk_tile"], meta["blk_first"], meta["blk_last"]
    )
    qtot = meta["qtot"]
    NSP = NS * NPASS
    NI16 = TB * 8
    STG = max(SHP, 2 * maxblk * 128)

    def gcum(spk):  # cumulative blocks through global sp-call spk
        return (spk // NSP) * TB + cum_sp[spk % NSP]

    nc = bacc.Bacc("TRN2", debug=False, num_swdge_queues=4)
    featT = nc.declare_dram_parameter("featT", [128, SHP], BF16, isOutput=False)
    idxs = nc.declare_dram_parameter("idxs", [16, NI16], I16, isOutput=False)
    drel = nc.declare_dram_parameter("drel", [128, TB], F32, isOutput=False)
    nrm = nc.declare_dram_parameter("nrm", [128, TB], F32, isOutput=False)
    iota = nc.declare_dram_parameter("iota", [128, 128], BF16, isOutput=False)
    w1 = nc.declare_dram_parameter("w1", [128, 128], BF16, isOutput=False)
    w2 = nc.declare_dram_parameter("w2", [128, 128], BF16, isOutput=False)
    b1 = nc.declare_dram_parameter("b1", [128, 1], F32, isOutput=False)
    a1 = nc.declare_dram_parameter("a1", [128, 1], F32, isOutput=False)
    b2bc = nc.declare_dram_parameter("b2bc", [128, 128], F32, isOutput=False)
    a2bc = nc.declare_dram_parameter("a2bc", [128, 128], F32, isOutput=False)
    out = nc.declare_dram_parameter("out", [SHP, D], BF16, isOutput=True)

    h_bounce = [nc.dram_tensor(f"h{l}_bounce", [SHP, D], BF16) for l in (1, 2)]
    h_table = [
        nc.dram_tensor(f"h{l}_table", [TBL, D], BF16, addr_space="Shared")
        for l in (1, 2)
    ]

    from contextlib import ExitStack
    with ExitStack() as ctx:
        ent = ctx.enter_context
        xbuf = ent(nc.sbuf_tensor("xbuf", [128, SHP], BF16))
        stage = ent(nc.sbuf_tensor("stage", [128, STG], BF16))
        idx_sb = ent(nc.sbuf_tensor("idx_sb", [128, NI16], I16))
        drel_sb = ent(nc.sbuf_tensor("drel_sb", [128, TB], F32))
        nrm_sb = ent(nc.sbuf_tensor("nrm_sb", [128, TB], F32))
        sring = ent(nc.sbuf_tensor("sring", [128, RING, 128], BF16))
        iota_sb = ent(nc.sbuf_tensor("iota_sb", [128, 128], BF16))
        w1_sb = ent(nc.sbuf_tensor("w1_sb", [128, 128], BF16))
        w2_sb = ent(nc.sbuf_tensor("w2_sb", [128, 128], BF16))
        b1_sb = ent(nc.sbuf_tensor("b1_sb", [128, 1], F32))
        a1_sb = ent(nc.sbuf_tensor("a1_sb", [128, 1], F32))
        b2bc_sb = ent(nc.sbuf_tensor("b2bc_sb", [128, 128], F32))
        a2bc_sb = ent(nc.sbuf_tensor("a2bc_sb", [128, 128], F32))
        tpos = ent(nc.sbuf_tensor("tpos", [128, 128], F32))
        tneg = ent(nc.sbuf_tensor("tneg", [128, 128], F32))
        ps_all = ent(nc.psum_tensor("ps_all", [128, 8, 512], F32))
        s_load = ent(nc.semaphore("s_load"))
        s_gat = [ent(nc.semaphore(f"s_ga{i}")) for i in range(8)]
        s_dve = ent(nc.semaphore("s_dve"))
        s_pe = ent(nc.semaphore("s_pe"))
        s_peg = ent(nc.semaphore("s_peg"))
        s_dveg = ent(nc.semaphore("s_dveg"))
        s_post = ent(nc.semaphore("s_post"))
        s_store = ent(nc.semaphore("s_store"))
        s_cc = ent(nc.semaphore("s_cc"))
        s_ch = ent(nc.semaphore("s_ch"))
        block = ent(nc.Block())

        def agg_ps(s, t):
            return ps_all[:, (s % 2) * 4 + t, 0:128]

        def gemm_ps(t):
            return ps_all[:, 4 + (t % 2), 0:128]
        stage3 = stage[:, : 2 * maxblk * 128].rearrange(
            "p (b f) -> p b f", f=128
        )

        def msg_ap(layer, gl):
            sp = int(blk_sp[gl])
            buf = (layer * NSP + sp) % 2
            loc = gl - int(bbase[sp * SUPER])
            return stage3[:, buf * maxblk + loc, :]

        chain = [0]  # DVE same-engine hazard chain counter
        pc = [0]     # completed posts (guards tpos/tneg WAR across tiles)

        def post(vector, layer, s):
            vector.wait_ge(s_pe, TB * layer + cum_sp[s * NPASS + NPASS - 1])
            for t in range(int(ntin[s])):
                gt = s * SUPER + t
                ps = agg_ps(s, t)
                xo = xbuf[:, gt * 128: (gt + 1) * 128]
                if pc[0] > 0:
                    vector.wait_ge(s_post, pc[0])
                if layer == 0:
                    vector.tensor_scalar(
                        tpos[:], ps, b1_sb[:, 0:1], 0.0,
                        op0=mybir.AluOpType.add, op1=mybir.AluOpType.max,
                    )
                    vector.tensor_scalar(
                        tneg[:], ps, b1_sb[:, 0:1], 0.0,
                        op0=mybir.AluOpType.add, op1=mybir.AluOpType.min,
                    ).then_inc(s_ch, 1)
                    chain[0] += 1
                    vector.wait_ge(s_ch, chain[0])
                    vector.tensor_scalar(
                        tneg[:], tneg[:], a1_sb[:, 0:1], None,
                        op0=mybir.AluOpType.mult,
                    ).then_inc(s_ch, 1)
                    chain[0] += 1
                    vector.wait_ge(s_ch, chain[0])
                    vector.tensor_tensor(
                        xo, tpos[:], tneg[:], op=mybir.AluOpType.add
                    ).then_inc(s_post, 1)
                    pc[0] += 1
                else:
                    vector.tensor_tensor(
                        tpos[:], ps, b2bc_sb[:], op=mybir.AluOpType.add
                    ).then_inc(s_ch, 1)
                    chain[0] += 1
                    vector.wait_ge(s_ch, chain[0])
                    vector.tensor_scalar(
                        tneg[:], tpos[:], 0.0, None, op0=mybir.AluOpType.min
                    ).then_inc(s_ch, 1)
                    chain[0] += 1
                    vector.wait_ge(s_ch, chain[0])
                    vector.tensor_scalar(
                        tpos[:], tpos[:], 0.0, None, op0=mybir.AluOpType.max
                    )
                    vector.tensor_tensor(
                        tneg[:], tneg[:], a2bc_sb[:], op=mybir.AluOpType.mult
                    ).then_inc(s_ch, 1)
                    chain[0] += 1
                    vector.wait_ge(s_ch, chain[0])
                    vector.tensor_tensor(
                        xo, tpos[:], tneg[:], op=mybir.AluOpType.add
                    ).then_inc(s_post, 1)
                    pc[0] += 1

        @block.sync
        def _(sync):
            for ap_d, ap_s in (
                (iota_sb[:], iota[:]), (w1_sb[:], w1[:]), (w2_sb[:], w2[:]),
                (b1_sb[:], b1[:]), (a1_sb[:], a1[:]),
                (b2bc_sb[:], b2bc[:]), (drel_sb[:], drel[:]),
                (a2bc_sb[:], a2bc[:]),
                (nrm_sb[:], nrm[:]), (xbuf[:, :SHP], featT[:]),
            ) + tuple(
                (idx_sb[16 * k:16 * (k + 1), :], idxs[:]) for k in range(8)
            ):
                sync.dma_start(out=ap_d, in_=ap_s).then_inc(s_load, 16)
            sync.wait_ge(s_dveg, NT)
            sync.dma_start(
                out=h_bounce[0].ap().rearrange("(t p) f -> p t f", p=128),
                in_=stage[:, : NT * 128].rearrange("p (t f) -> p t f", f=128),
            ).then_inc(s_store, 16)
            sync.wait_ge(s_dveg, 2 * NT)
            sync.dma_start(
                out=h_bounce[1].ap().rearrange("(t p) f -> p t f", p=128),
                in_=stage[:, : NT * 128].rearrange("p (t f) -> p t f", f=128),
            ).then_inc(s_store, 16)
            sync.wait_ge(s_post, 2 * NT)
            sync.dma_start(
                out=out.ap().rearrange("(t p) f -> p t f", p=128),
                in_=xbuf[:, : NT * 128].rearrange("p (t f) -> p t f", f=128),
            ).then_inc(s_store, 16)
            sync.wait_ge(s_store, 48)

        @block.gpsimd
        def _(gpsimd):
            gpsimd.load_library(library_config.mlp)
            for layer in range(2):
                gpsimd.wait_ge(s_store, 16 * (layer + 1))
                gpsimd.collective_compute(
                    "AllGather",
                    mybir.AluOpType.bypass,
                    replica_groups=[list(range(C))],
                    ins=[h_bounce[layer][:]],
                    outs=[h_table[layer][:]],
                ).then_inc(s_cc)
                gpsimd.wait_ge(s_cc, layer + 1)
                for k, (s, p, fb, nbl, q, qs) in enumerate(calls):
                    sp = s * NPASS + p
                    spk = layer * NSP + sp
                    kk = layer * len(calls) + k
                    if spk >= 2 and fb == int(bbase[sp * SUPER]):
                        gpsimd.wait_ge(s_pe, gcum(spk - 2))
                    buf = spk % 2
                    loc = fb - int(bbase[sp * SUPER])
                    gpsimd.dma_gather(
                        stage3[:, buf * maxblk + loc: buf * maxblk + loc + nbl, :],
                        h_table[layer][p * PR: min((p + 1) * PR, TBL), :],
                        idx_sb[:, fb * 8: (fb + nbl) * 8],
                        nbl * 128,
                        nbl * 128,
                        D,
                        queue_num=q,
                    ).then_inc(s_gat[kk % 8], 16)

        @block.vector
        def _(vector):
            vector.wait_ge(s_load, 288)
            for t in range(NT):
                vector.wait_ge(s_peg, t + 1)
                vector.tensor_copy(
                    stage[:, t * 128: (t + 1) * 128], gemm_ps(t)
                ).then_inc(s_dveg, 1)
            for layer in range(2):
                g0 = TB * layer
                for s in range(NS):
                    for p in range(NPASS):
                        sp = s * NPASS + p
                        fb0 = int(bbase[sp * SUPER])
                        nb = int(B[s, p].sum())
                        for gl in range(fb0, fb0 + nb):
                            g = g0 + gl
                            if gl % GRP == 0 and g >= RING:
                                vector.wait_ge(s_pe, g - RING + GRP)
                            vector.tensor_scalar(
                                sring[:, g % RING, :],
                                iota_sb[:],
                                drel_sb[:, gl: gl + 1],
                                nrm_sb[:, gl: gl + 1],
                                op0=mybir.AluOpType.is_equal,
                                op1=mybir.AluOpType.mult,
                            ).then_inc(s_dve, 1)
                    if s >= 1:
                        post(vector, layer, s - 1)
                post(vector, layer, NS - 1)
                if layer == 0:
                    for t in range(NT):
                        vector.wait_ge(s_peg, NT + t + 1)
                        vector.tensor_copy(
                            stage[:, t * 128: (t + 1) * 128], gemm_ps(t)
                        ).then_inc(s_dveg, 1)

        @block.tensor
        def _(tensor):
            tensor.wait_ge(s_load, 288)
            for t in range(NT):
                if t >= 2:
                    tensor.wait_ge(s_dveg, t - 1)
                tensor.matmul(
                    gemm_ps(t),
                    xbuf[:, t * 128: (t + 1) * 128],
                    w1_sb[:],
                    start=True, stop=True,
                ).then_inc(s_peg, 1)
            for layer in range(2):
                g0 = TB * layer
                for k, (s, p, fb, nbl, q, qs) in enumerate(calls):
                    kk = layer * len(calls) + k
                    tensor.wait_ge(s_gat[kk % 8], 16 * (kk // 8 + 1))
                    for gl in range(fb, fb + nbl):
                        g = g0 + gl
                        if gl % GRP == 0:
                            tensor.wait_ge(s_dve, min(g + GRP, g0 + TB))
                        t = int(blk_tile[gl])
                        ps = agg_ps(s, t)
                        m = msg_ap(layer, gl)
                        sr = sring[:, g % RING, :]
                        if layer == 0:
                            mm = tensor.matmul(
                                ps, m, sr,
                                start=bool(blk_first[gl]),
                                stop=bool(blk_last[gl]),
                                skip_group_check=True,
                            )
                        else:
                            mm = tensor.matmul(
                                ps, sr, m,
                                start=bool(blk_first[gl]),
                                stop=bool(blk_last[gl]),
                                skip_group_check=True,
                            )
                        mm.then_inc(s_pe, 1)
                if layer == 0:
                    for t in range(NT):
                        if t == 0:
                            tensor.wait_ge(s_post, NT)
                        if t >= 2:
                            tensor.wait_ge(s_dveg, NT + t - 1)
                        tensor.matmul(
                            gemm_ps(t),
                            xbuf[:, t * 128: (t + 1) * 128],
                            w2_sb[:],
                            start=True, stop=True,
                        ).then_inc(s_peg, 1)

    nc.compile()
    return nc


def prepare(features, edge_index, edge_weight, W1, b1, a1, W2, b2, a2):
    N, Dd = features.shape
    assert Dd == D
    src = np.asarray(edge_index[0], dtype=np.int64)
    dst = np.asarray(edge_index[1], dtype=np.int64)
    w = np.asarray(edge_weight, dtype=np.float32)

    deg = (np.bincount(dst, weights=w.astype(np.float64), minlength=N) + 1.0)
    dis = (1.0 / np.sqrt(deg)).astype(np.float32)
    norm = dis[src] * w * dis[dst]
    allsrc = np.concatenate([src, np.arange(N, dtype=np.int64)])
    alldst = np.concatenate([dst, np.arange(N, dtype=np.int64)])
    allnorm = np.concatenate([norm, (dis * dis).astype(np.float32)])

    meta, idx_f, dr_f, nm_f = _schedule(N, allsrc, alldst, allnorm)
    SH, SHP, TB = meta["SH"], meta["SHP"], meta["TB"]

    idx_w = idx_f.reshape(C, TB * 8, 16).transpose(0, 2, 1).copy()
    dr_w = dr_f.reshape(C, TB, 128).transpose(0, 2, 1).copy()
    nm_w = nm_f.reshape(C, TB, 128).transpose(0, 2, 1).copy()

    featT = np.zeros((C, 128, SHP), dtype=bfloat16)
    fpad = np.asarray(features, dtype=np.float32)
    for c in range(C):
        featT[c, :, :SH] = fpad[c * SH:(c + 1) * SH].T.astype(bfloat16)

    iota = np.tile(np.arange(128, dtype=np.float32)[None, :],
                   (128, 1)).astype(bfloat16)
    in_maps = []
    for c in range(C):
        in_maps.append(dict(
            featT=featT[c], idxs=idx_w[c], drel=dr_w[c], nrm=nm_w[c],
            iota=iota,
            w1=np.asarray(W1, np.float32).astype(bfloat16),
            w2=np.asarray(W2, np.float32).astype(bfloat16),
            b1=np.asarray(b1, np.float32).reshape(128, 1),
            a1=np.asarray(a1, np.float32).reshape(128, 1),
            b2bc=np.tile(np.asarray(b2, np.float32)[None, :], (128, 1)),
            a2bc=np.tile(np.asarray(a2, np.float32)[None, :], (128, 1)),
        ))
    return meta, in_maps


def kernel(features, edge_index, edge_weight, W1, b1, a1, W2, b2, a2):
    meta, in_maps = prepare(
        features, edge_index, edge_weight, W1, b1, a1, W2, b2, a2
    )
    nc = build_program(meta)
    res = run_bass_kernel_spmd(nc, in_maps, core_ids=list(range(C))).results
    SH = meta["SH"]
    return np.concatenate(
        [r["out"][:SH].astype(np.float32) for r in res], axis=0
    )



# revision 22
# speedup vs baseline: 3.0281x; 2.9935x over previous
"""Self-contained Trainium2 Bass kernel for the 2-layer GCN encoder.

kernel(**inputs) takes FULL inputs (features [100000,128] f32,
edge_index [2,1600000] int, edge_weight [1600000] f32, W1,b1,a1,W2,b2,a2)
and returns the FULL [100000,128] f32 output, running on 8 NeuronCores.

Strategy (dst-sharded message passing, re-associated GEMMs, bf16):
  GCN layer = PReLU(A_norm @ X @ W + b). We aggregate FIRST, then GEMM:
  layer1 aggregates the RAW features (table staged in DRAM up front -> no
  collective before layer-1 aggregation); layer2 aggregates the layer-1
  output (one AllGather mid-kernel).
  - nodes sharded across 8 cores (SH each, padded to NT*128)
  - per layer: dst-sharded segment-sum via dma_gather of 256B bf16 rows +
    one-hot*norm S matrices (fused is_equal/mult tensor_scalar) + PE
    matmul in PSUM: ps[f, d_tile] = sum_e msgs[e,f] * S[e,d]
  - per super-tile: DVE copies psum -> xagg (bf16), PE runs the layer GEMM
    (xagg_tile @ W), DVE applies bias+PReLU with row-broadcast b/a tiles
  - edges grouped per (dst-super-tile, src-pass); per-(tile,pass) block
    counts baked in (max over cores -> SPMD-uniform schedule)
"""
import os
os.environ.setdefault("NEURON_RT_RESET_CORES", "1")

import sys
sys.path.insert(0, "/opt/trn_rl_repo")
import numpy as np
from ml_dtypes import bfloat16
from concourse import bacc, mybir, library_config
from concourse.bass_utils import run_bass_kernel_spmd

F32 = mybir.dt.float32
BF16 = mybir.dt.bfloat16
I16 = mybir.dt.int16

C = 8            # cores
D = 128          # feature dim
NPASS = 4        # src-range passes (int16 gather indices)
SUPER = 4        # dst tiles per super-tile (one PSUM bank per tile)
MAXIDX = 1024    # max indices per dma_gather call (SWDGE ring limit)
RING = 64        # S-tile ring slots
GRP = 8          # S-ring sync granularity (blocks)


def _schedule(N, src, dst, norm):
    """Group edges into an SPMD-uniform static schedule."""
    SH = N // C
    NT = (SH + 127) // 128          # dst tiles per core
    SHP = NT * 128
    NS = (NT + SUPER - 1) // SUPER  # super tiles
    TBL = SHP * C
    PR = (TBL + NPASS - 1) // NPASS
    PR = ((PR + 127) // 128) * 128  # pass rows (aligned)
    assert PR <= 32767

    core = dst // SH
    dloc = dst - core * SH
    tile = dloc // 128
    srel = (dloc % 128).astype(np.float32)
    sup = tile // SUPER
    tin = tile % SUPER              # tile index within super
    tbl = (src // SH) * SHP + (src % SH)
    pas = tbl // PR
    prel = (tbl % PR).astype(np.int16)

    ntin = np.minimum(SUPER, NT - np.arange(NS) * SUPER)  # tiles in super s

    key = ((core * NS + sup) * NPASS + pas) * SUPER + tin
    cnt = np.bincount(key, minlength=C * NS * NPASS * SUPER).reshape(
        C, NS, NPASS, SUPER
    )
    B = np.maximum(1, -(-cnt.max(axis=0) // 128))         # [NS, NPASS, SUPER]
    for s in range(NS):
        B[s, :, ntin[s]:] = 0
    Bf = B.reshape(-1)
    bbase = np.concatenate([[0], np.cumsum(Bf)]).astype(np.int64)
    TB = int(bbase[-1])                                   # blocks per layer

    # per-edge slot: blockbase(s,p,t)*128 + rank within (core,s,p,t) group
    ekey = (sup * NPASS + pas) * SUPER + tin
    order = np.lexsort((ekey, core))
    sc, se = core[order], ekey[order]
    gkey = sc * (NS * NPASS * SUPER) + se
    _, first = np.unique(gkey, return_index=True)
    starts = np.zeros(len(gkey), dtype=np.int64)
    starts[first] = first
    starts = np.maximum.accumulate(starts)
    rank = np.arange(len(gkey)) - starts
    slot = bbase[se] * 128 + rank

    idx_f = np.zeros((C, TB * 128), dtype=np.int16)
    dr_f = np.zeros((C, TB * 128), dtype=np.float32)
    nm_f = np.zeros((C, TB * 128), dtype=np.float32)
    idx_f[sc, slot] = prel[order]
    dr_f[sc, slot] = srel[order]
    nm_f[sc, slot] = norm[order]

    # gather sub-calls: per (s,p), chunks of <= MAXIDX/128 blocks
    calls = []   # (s, p, fb(layer-block), nbl, queue, qseq)
    qcnt = [0, 0, 0, 0]
    for s in range(NS):
        for p in range(NPASS):
            nb = int(B[s, p].sum())
            fb0 = int(bbase[(s * NPASS + p) * SUPER])
            off = 0
            while off < nb:
                n = min(nb - off, MAXIDX // 128)
                q = len(calls) % 4
                qcnt[q] += 1
                calls.append((s, p, fb0 + off, n, q, qcnt[q]))
                off += n
    maxblk = max(int(B[s, p].sum()) for s in range(NS) for p in range(NPASS))

    # cumulative blocks through sp
    cum_sp = {}
    acc = 0
    blk_sp = np.zeros(TB, dtype=np.int64)
    for s in range(NS):
        for p in range(NPASS):
            sp = s * NPASS + p
            fb0 = int(bbase[sp * SUPER])
            nb = int(B[s, p].sum())
            blk_sp[fb0:fb0 + nb] = sp
            acc += nb
            cum_sp[sp] = acc

    # block -> (tile-in-super, is_first, is_last)
    blk_tile = np.zeros(TB, dtype=np.int64)
    blk_first = np.zeros(TB, dtype=bool)
    blk_last = np.zeros(TB, dtype=bool)
    for s in range(NS):
        for p in range(NPASS):
            for t in range(int(ntin[s])):
                b0 = int(bbase[(s * NPASS + p) * SUPER + t])
                nb = int(B[s, p, t])
                blk_tile[b0:b0 + nb] = t
                if p == 0:
                    blk_first[b0] = True
                if p == NPASS - 1:
                    blk_last[b0 + nb - 1] = True

    return dict(
        N=N, SH=SH, NT=NT, SHP=SHP, NS=NS, TBL=TBL, PR=PR, TB=TB,
        B=B, bbase=bbase, ntin=ntin, calls=calls, maxblk=maxblk,
        blk_sp=blk_sp, cum_sp=cum_sp, blk_tile=blk_tile,
        blk_first=blk_first, blk_last=blk_last,
    ), idx_f, dr_f, nm_f


def build_program(meta, debug_dump=False):
    NT, SHP, NS, TBL, PR, TB = (
        meta["NT"], meta["SHP"], meta["NS"], meta["TBL"], meta["PR"], meta["TB"]
    )
    B, bbase, ntin, calls, maxblk = (
        meta["B"], meta["bbase"], meta["ntin"], meta["calls"], meta["maxblk"]
    )
    blk_sp, cum_sp = meta["blk_sp"], meta["cum_sp"]
    blk_tile, blk_first, blk_last = (
        meta["blk_tile"], meta["blk_first"], meta["blk_last"]
    )
    NSP = NS * NPASS
    NI16 = TB * 8
    STG = 2 * maxblk * 128

    def gcum(spk):  # cumulative blocks through global sp-chunk spk
        return (spk // NSP) * TB + cum_sp[spk % NSP]

    def agg_through(layer, s):  # agg matmuls done through super s, layer
        return TB * layer + cum_sp[s * NPASS + NPASS - 1]

    nc = bacc.Bacc("TRN2", debug=False, num_swdge_queues=4)
    xtab = nc.declare_dram_parameter("xtab", [TBL, D], BF16, isOutput=False)
    idxs = nc.declare_dram_parameter("idxs", [128, NI16], I16, isOutput=False)
    drel = nc.declare_dram_parameter("drel", [128, TB], F32, isOutput=False)
    nrm = nc.declare_dram_parameter("nrm", [128, TB], F32, isOutput=False)
    iota = nc.declare_dram_parameter("iota", [128, 128], BF16, isOutput=False)
    w1 = nc.declare_dram_parameter("w1", [128, 128], BF16, isOutput=False)
    w2 = nc.declare_dram_parameter("w2", [128, 128], BF16, isOutput=False)
    b1bc = nc.declare_dram_parameter("b1bc", [128, 128], F32, isOutput=False)
    a1bc = nc.declare_dram_parameter("a1bc", [128, 128], F32, isOutput=False)
    b2bc = nc.declare_dram_parameter("b2bc", [128, 128], F32, isOutput=False)
    a2bc = nc.declare_dram_parameter("a2bc", [128, 128], F32, isOutput=False)
    out = nc.declare_dram_parameter("out", [SHP, D], BF16, isOutput=True)
    if debug_dump:
        xaggdump = nc.declare_dram_parameter(
            "xaggdump", [128, SHP], BF16, isOutput=True)
        h1dump = nc.declare_dram_parameter(
            "h1dump", [SHP, D], BF16, isOutput=True)

    h_bounce = nc.dram_tensor("h_bounce", [SHP, D], BF16)
    h_table = nc.dram_tensor("h_table", [TBL, D], BF16, addr_space="Shared")
    xtab_int = nc.dram_tensor("xtab_int", [TBL, D], BF16)

    from contextlib import ExitStack
    with ExitStack() as ctx:
        ent = ctx.enter_context
        xagg = ent(nc.sbuf_tensor("xagg", [128, SHP], BF16))
        hstage = ent(nc.sbuf_tensor("hstage", [128, SHP], BF16))
        stage = ent(nc.sbuf_tensor("stage", [128, STG], BF16))
        idx_sb = ent(nc.sbuf_tensor("idx_sb", [128, NI16], I16))
        drel_sb = ent(nc.sbuf_tensor("drel_sb", [128, TB], F32))
        nrm_sb = ent(nc.sbuf_tensor("nrm_sb", [128, TB], F32))
        sring = ent(nc.sbuf_tensor("sring", [128, RING, 128], BF16))
        iota_sb = ent(nc.sbuf_tensor("iota_sb", [128, 128], BF16))
        w1_sb = ent(nc.sbuf_tensor("w1_sb", [128, 128], BF16))
        w2_sb = ent(nc.sbuf_tensor("w2_sb", [128, 128], BF16))
        b1bc_sb = ent(nc.sbuf_tensor("b1bc_sb", [128, 128], F32))
        a1bc_sb = ent(nc.sbuf_tensor("a1bc_sb", [128, 128], F32))
        b2bc_sb = ent(nc.sbuf_tensor("b2bc_sb", [128, 128], F32))
        a2bc_sb = ent(nc.sbuf_tensor("a2bc_sb", [128, 128], F32))
        tpos = ent(nc.sbuf_tensor("tpos", [128, 128], F32))
        tneg = ent(nc.sbuf_tensor("tneg", [128, 128], F32))
        ps_all = ent(nc.psum_tensor("ps_all", [128, 8, 512], F32))
        s_load = ent(nc.semaphore("s_load"))
        s_gat = [ent(nc.semaphore(f"s_ga{i}")) for i in range(8)]
        s_dve = ent(nc.semaphore("s_dve"))
        s_pe = ent(nc.semaphore("s_pe"))      # agg matmuls done
        s_peg = ent(nc.semaphore("s_peg"))    # gemm matmuls done
        s_dveg = ent(nc.semaphore("s_dveg"))  # xagg copies done
        s_post = ent(nc.semaphore("s_post"))  # post tiles done
        s_store = ent(nc.semaphore("s_store"))
        s_cc = ent(nc.semaphore("s_cc"))
        s_xt = ent(nc.semaphore("s_xt"))
        s_dbg = ent(nc.semaphore("s_dbg"))
        s_ch = ent(nc.semaphore("s_ch"))
        block = ent(nc.Block())

        def agg_ps(s, t):
            return ps_all[:, (s % 2) * 4 + t, 0:128]

        stage3 = stage.rearrange("p (b f) -> p b f", f=128)

        def msg_ap(layer, gl):
            sp = int(blk_sp[gl])
            buf = (layer * NSP + sp) % 2
            loc = gl - int(bbase[sp * SUPER])
            return stage3[:, buf * maxblk + loc, :]

        chain = [0]   # DVE same-engine hazard chain counter
        pc = [0]      # completed posts (tpos/tneg WAR guard)
        nload = 10  # dma loads in sync preamble

        def xagg_copy(vector, layer, s):
            """PSUM agg result -> xagg bf16 (frees agg banks for gemm)."""
            vector.wait_ge(s_pe, agg_through(layer, s))
            for t in range(int(ntin[s])):
                gt = s * SUPER + t
                vector.tensor_copy(
                    xagg[:, gt * 128:(gt + 1) * 128], agg_ps(s, t)
                ).then_inc(s_dveg, 1)

        def post(vector, layer, s):
            """bias + PReLU on gemm psum -> hstage bf16."""
            bbc = b1bc_sb if layer == 0 else b2bc_sb
            abc = a1bc_sb if layer == 0 else a2bc_sb
            for t in range(int(ntin[s])):
                gt = s * SUPER + t
                vector.wait_ge(s_peg, layer * NT + gt + 1)
                if pc[0] > 0:
                    vector.wait_ge(s_post, pc[0])
                pc[0] += 1
                ps = agg_ps(s, t)
                vector.tensor_tensor(
                    tpos[:], ps, bbc[:], op=mybir.AluOpType.add
                ).then_inc(s_ch, 1)
                chain[0] += 1
                vector.wait_ge(s_ch, chain[0])
                vector.tensor_scalar(
                    tneg[:], tpos[:], 0.0, None, op0=mybir.AluOpType.min
                ).then_inc(s_ch, 1)
                chain[0] += 1
                vector.wait_ge(s_ch, chain[0])
                vector.tensor_scalar(
                    tpos[:], tpos[:], 0.0, None, op0=mybir.AluOpType.max
                )
                vector.tensor_tensor(
                    tneg[:], tneg[:], abc[:], op=mybir.AluOpType.mult
                ).then_inc(s_ch, 1)
                chain[0] += 1
                vector.wait_ge(s_ch, chain[0])
                vector.tensor_tensor(
                    hstage[:, gt * 128:(gt + 1) * 128], tpos[:], tneg[:],
                    op=mybir.AluOpType.add,
                ).then_inc(s_post, 1)

        @block.sync
        def _(sync):
            QTR = TBL // NPASS
            for p in range(NPASS):
                sync.dma_start(
                    out=xtab_int[p * QTR:(p + 1) * QTR, :],
                    in_=xtab[p * QTR:(p + 1) * QTR, :],
                ).then_inc(s_xt, 16)
            for ap_d, ap_s in (
                (iota_sb[:], iota[:]), (w1_sb[:], w1[:]), (w2_sb[:], w2[:]),
                (b1bc_sb[:], b1bc[:]), (a1bc_sb[:], a1bc[:]),
                (b2bc_sb[:], b2bc[:]), (a2bc_sb[:], a2bc[:]),
                (drel_sb[:], drel[:]), (nrm_sb[:], nrm[:]),
            ) + ((idx_sb[:], idxs[:]),):
                sync.dma_start(out=ap_d, in_=ap_s).then_inc(s_load, 16)
            if debug_dump:
                sync.wait_ge(s_dveg, NT)  # layer-0 xagg copies done
                sync.dma_start(out=xaggdump.ap(), in_=xagg[:, :SHP]
                               ).then_inc(s_dbg, 16)
            sync.wait_ge(s_post, NT)
            if debug_dump:
                sync.dma_start(out=h1dump.ap().rearrange(
                    "(t p) f -> p t f", p=128),
                    in_=hstage[:, : NT * 128].rearrange(
                        "p (t f) -> p t f", f=128),
                ).then_inc(s_dbg, 16)
            sync.dma_start(
                out=h_bounce.ap().rearrange("(t p) f -> p t f", p=128),
                in_=hstage[:, : NT * 128].rearrange("p (t f) -> p t f", f=128),
            ).then_inc(s_store, 16)
            sync.wait_ge(s_post, 2 * NT)
            sync.dma_start(
                out=out.ap().rearrange("(t p) f -> p t f", p=128),
                in_=hstage[:, : NT * 128].rearrange("p (t f) -> p t f", f=128),
            ).then_inc(s_store, 16)
            if debug_dump:
                sync.wait_ge(s_store, 32)
            else:
                sync.wait_ge(s_store, 32)

        @block.gpsimd
        def _(gpsimd):
            gpsimd.load_library(library_config.mlp)
            gpsimd.wait_ge(s_load, nload * 16)
            for layer in range(2):
                if layer == 1:
                    gpsimd.wait_ge(s_store, 16)
                    gpsimd.collective_compute(
                        "AllGather",
                        mybir.AluOpType.bypass,
                        replica_groups=[list(range(C))],
                        ins=[h_bounce[:]],
                        outs=[h_table[:]],
                    ).then_inc(s_cc)
                    gpsimd.wait_ge(s_cc, 1)
                table = xtab_int if layer == 0 else h_table
                for k, (s, p, fb, nbl, q, qs) in enumerate(calls):
                    sp = s * NPASS + p
                    spk = layer * NSP + sp
                    kk = layer * len(calls) + k
                    if layer == 0 and fb == int(bbase[sp * SUPER]):
                        gpsimd.wait_ge(s_xt, 4 * 16)
                    if spk >= 2 and fb == int(bbase[sp * SUPER]):
                        gpsimd.wait_ge(s_pe, gcum(spk - 2))
                    buf = spk % 2
                    loc = fb - int(bbase[sp * SUPER])
                    gpsimd.dma_gather(
                        stage3[:, buf * maxblk + loc: buf * maxblk + loc + nbl, :],
                        table[p * PR: min((p + 1) * PR, TBL), :],
                        idx_sb[:, fb * 8: (fb + nbl) * 8],
                        nbl * 128,
                        nbl * 128,
                        D,
                        queue_num=q,
                    ).then_inc(s_gat[kk % 8], 16)

        @block.vector
        def _(vector):
            vector.wait_ge(s_load, nload * 16)
            for layer in range(2):
                g0 = TB * layer
                if debug_dump and layer == 1:
                    vector.wait_ge(s_dbg, 32)
                for s in range(NS):
                    if s >= 1:
                        xagg_copy(vector, layer, s - 1)
                    for p in range(NPASS):
                        sp = s * NPASS + p
                        fb0 = int(bbase[sp * SUPER])
                        nb = int(B[s, p].sum())
                        for gl in range(fb0, fb0 + nb):
                            g = g0 + gl
                            if gl % GRP == 0 and g >= RING:
                                vector.wait_ge(s_pe, g - RING + GRP)
                            vector.tensor_scalar(
                                sring[:, g % RING, :],
                                iota_sb[:],
                                drel_sb[:, gl: gl + 1],
                                nrm_sb[:, gl: gl + 1],
                                op0=mybir.AluOpType.is_equal,
                                op1=mybir.AluOpType.mult,
                            ).then_inc(s_dve, 1)
                xagg_copy(vector, layer, NS - 1)
                for s in range(NS):
                    post(vector, layer, s)

        @block.tensor
        def _(tensor):
            tensor.wait_ge(s_load, nload * 16)
            w_sb = [w1_sb, w2_sb]
            for layer in range(2):
                g0 = TB * layer
                for k, (s, p, fb, nbl, q, qs) in enumerate(calls):
                    kk = layer * len(calls) + k
                    # agg(s) reuses agg(s-2) banks: xagg copies must be done
                    if p == 0 and fb == int(bbase[(s * NPASS) * SUPER]) \
                            and s >= 2:
                        sg = s - 2
                        tensor.wait_ge(
                            s_dveg,
                            layer * NT + sg * SUPER + int(ntin[sg]))
                    tensor.wait_ge(s_gat[kk % 8], 16 * (kk // 8 + 1))
                    sup_end = g0 + cum_sp[s * NPASS + NPASS - 1]
                    for gl in range(fb, fb + nbl):
                        g = g0 + gl
                        if gl == fb or gl % GRP == 0:
                            tensor.wait_ge(
                                s_dve,
                                min(g + GRP - gl % GRP, sup_end))
                        t = int(blk_tile[gl])
                        tensor.matmul(
                            agg_ps(s, t), msg_ap(layer, gl),
                            sring[:, g % RING, :],
                            start=bool(blk_first[gl]),
                            stop=bool(blk_last[gl]),
                            skip_group_check=True,
                        ).then_inc(s_pe, 1)
                # layer gemms (all agg copies must be done: banks shared)
                for sg in range(NS):
                    if sg >= 2:
                        tensor.wait_ge(
                            s_post,
                            layer * NT + (sg - 2) * SUPER
                            + int(ntin[sg - 2]))
                    for t in range(int(ntin[sg])):
                        gt = sg * SUPER + t
                        tensor.wait_ge(s_dveg, layer * NT + NT)
                        tensor.matmul(
                            agg_ps(sg, t),
                            xagg[:, gt * 128:(gt + 1) * 128],
                            w_sb[layer][:],
                            start=True, stop=True,
                        ).then_inc(s_peg, 1)

    nc.compile()
    return nc


def prepare(features, edge_index, edge_weight, W1, b1, a1, W2, b2, a2):
    N, Dd = features.shape
    assert Dd == D
    src = np.asarray(edge_index[0], dtype=np.int64)
    dst = np.asarray(edge_index[1], dtype=np.int64)
    w = np.asarray(edge_weight, dtype=np.float32)

    deg = (np.bincount(dst, weights=w.astype(np.float64), minlength=N) + 1.0)
    dis = (1.0 / np.sqrt(deg)).astype(np.float32)
    norm = dis[src] * w * dis[dst]
    allsrc = np.concatenate([src, np.arange(N, dtype=np.int64)])
    alldst = np.concatenate([dst, np.arange(N, dtype=np.int64)])
    allnorm = np.concatenate([norm, (dis * dis).astype(np.float32)])

    meta, idx_f, dr_f, nm_f = _schedule(N, allsrc, alldst, allnorm)
    SH, SHP, TB, TBL = meta["SH"], meta["SHP"], meta["TB"], meta["TBL"]

    idx_w = np.tile(
        idx_f.reshape(C, TB * 8, 16).transpose(0, 2, 1), (1, 8, 1)).copy()
    dr_w = dr_f.reshape(C, TB, 128).transpose(0, 2, 1).copy()
    nm_w = nm_f.reshape(C, TB, 128).transpose(0, 2, 1).copy()

    xtab = np.zeros((TBL, D), dtype=bfloat16)
    fall = np.asarray(features, dtype=np.float32)
    for c in range(C):
        xtab[c * SHP: c * SHP + SH] = fall[c * SH:(c + 1) * SH].astype(
            bfloat16)

    iota = np.tile(np.arange(128, dtype=np.float32)[None, :],
                   (128, 1)).astype(bfloat16)

    def bc(v):
        return np.tile(np.asarray(v, np.float32)[None, :], (128, 1))

    in_maps = []
    for c in range(C):
        in_maps.append(dict(
            xtab=xtab, idxs=idx_w[c], drel=dr_w[c], nrm=nm_w[c],
            iota=iota,
            w1=np.asarray(W1, np.float32).astype(bfloat16),
            w2=np.asarray(W2, np.float32).astype(bfloat16),
            b1bc=bc(b1), a1bc=bc(a1), b2bc=bc(b2), a2bc=bc(a2),
        ))
    return meta, in_maps


def kernel(features, edge_index, edge_weight, W1, b1, a1, W2, b2, a2):
    meta, in_maps = prepare(
        features, edge_index, edge_weight, W1, b1, a1, W2, b2, a2
    )
    nc = build_program(meta)
    res = run_bass_kernel_spmd(nc, in_maps, core_ids=list(range(C))).results
    SH = meta["SH"]
    return np.concatenate(
        [r["out"][:SH].astype(np.float32) for r in res], axis=0
    )
